# revision 13
# baseline (speedup 1.0000x reference)
"""Trainium2 Bass kernel for nn_MEGNet_State_876173328941.

MEGNet state update: u_e = scatter_mean(edge_attr, batch[edge_index[0]], B),
u_v = scatter_mean(x, batch, B), comb = [u_e, u_v, state], then a 3-layer MLP
(96->32->32->32) with training-mode BatchNorm over the batch dim.

v4 design: transposed streaming layout, dual-engine free-dim reduction,
two-phase AllGather (first half hidden under the stream).
  - Host folds the 1/count division into the data, casts to fp16, and packs
    each core's stream TRANSPOSED: partition p = 32*b + feat where b is the
    graph's block within its quad (4 graphs per quad), free dim = row index.
    Graph rows are contiguous column ranges, zero-padded to a shared
    cross-core schedule.
  - Device streams [128, CW] fp16 chunks and segment-reduces along the free
    dim with Vector (tensor_reduce) and Scalar (activation accum_out) in
    parallel; greedy cost-balanced piece assignment. Node stream goes first
    so its many small reduces hide under the edge DMA.
  - Grouped means are cast+moved straight into the collective input with
    gpsimd SBUF->DRAM casting DMAs. Quads 0..15 AllGather mid-stream
    (hidden); quads 16..31 AllGather at the end (pays the ~20us CC floor
    once). The tiny MLP+BatchNorm runs redundantly on every core in
    [feat, graph] layout, with BatchNorm restructured as E[h^2]-m^2 so the
    Scalar engine stays on one activation table set.
"""

import sys

sys.path.insert(0, "/opt/trn_rl_repo")

import numpy as np

import concourse.bacc as bacc
import concourse.tile as tile
from concourse import mybir
from concourse.bass_utils import run_bass_kernel_spmd

DIM = 32
B = 1024
N_CORES = 8
SEGS = 128          # graphs per core
NQ = SEGS // 4      # quads (groups of 4 graphs) per core
NH = NQ // 2        # quads per gather phase
CW = 16384          # stream columns per DMA chunk
ALIGN = 64
EPS = 1e-5

_CACHE = {}


def _plan(ecnt, ncnt):
    """Balanced graph->core assignment plus shared per-quad column widths."""
    w = ecnt + ncnt

    order_desc = np.argsort(-w, kind="stable")
    load = np.zeros(N_CORES, dtype=np.int64)
    nseg = np.zeros(N_CORES, dtype=np.int64)
    assign = np.zeros(B, dtype=np.int64)
    for s in order_desc:
        open_cores = np.where(nseg < SEGS)[0]
        k = open_cores[np.argmin(load[open_cores])]
        assign[s] = k
        load[k] += w[s]
        nseg[k] += 1

    order = np.zeros((N_CORES, SEGS), dtype=np.int64)   # rank -> global seg
    rank_of = np.zeros(B, dtype=np.int64)
    for k in range(N_CORES):
        segs_k = np.where(assign == k)[0]
        segs_k = segs_k[np.argsort(-w[segs_k], kind="stable")]
        order[k] = segs_k
        rank_of[segs_k] = np.arange(SEGS)

    def gsched(cnt):
        c = cnt[order].reshape(N_CORES, NQ, 4)     # [core, quad, block]
        m = c.max(axis=(0, 2))                     # [NQ]
        return ((m + ALIGN - 1) // ALIGN * ALIGN).astype(np.int64)

    gsched_e = gsched(ecnt)
    gsched_n = gsched(ncnt)

    # gathered local col l = 64*half + 16*b + (q % 16) for rank r = 4*q + b
    p_global = np.zeros(N_CORES * SEGS, dtype=np.int64)
    for k in range(N_CORES):
        for r in range(SEGS):
            q, bq = r // 4, r % 4
            ll = 64 * (q // NH) + 16 * bq + (q % NH)
            p_global[k * SEGS + ll] = order[k, r]
    return assign, rank_of, gsched_e, gsched_n, p_global


def _col_plan(gs):
    """Column bases, padded width, and chunk-relative reduce pieces.

    Returns (base[NQ+1], W_pad, chunks, pieces) where chunks is a list of
    (col0, width) and pieces is a list of (chunk_idx, lo, hi, group, nth).
    """
    base = np.zeros(NQ + 1, dtype=np.int64)
    np.cumsum(gs, out=base[1:])
    W = int(base[-1])
    W_pad = (W + 511) // 512 * 512
    chunks = []
    c0 = 0
    while c0 < W_pad:
        cw = min(CW, W_pad - c0)
        chunks.append((c0, cw))
        c0 += cw
    pieces = []
    for g in range(NQ):
        lo, hi = int(base[g]), int(base[g + 1])
        nth = 0
        for ci, (c0, cw) in enumerate(chunks):
            a, b_ = max(lo, c0), min(hi, c0 + cw)
            if a < b_:
                pieces.append((ci, a - c0, b_ - c0, g, nth))
                nth += 1
        assert nth >= 1
    return base, W_pad, chunks, pieces


# modeled ns cost per reduce piece, per engine
def _eng_cost(eng, fd):
    if eng == 0:     # Vector (DVE)
        return (58 + fd) / 0.96
    return (352 + fd) / 1.2  # Scalar (ACT)


def _build_nc(plan_pack):
    (We, chunks_e, pieces_e), (Wn, chunks_n, pieces_n) = plan_pack
    nc = bacc.Bacc("TRN2", target_bir_lowering=False, debug=False,
                   enable_asserts=False, num_devices=N_CORES)
    f16 = mybir.dt.float16
    f32 = mybir.dt.float32

    ev = nc.declare_dram_parameter("ev", [128, We], f16, isOutput=False)
    nv = nc.declare_dram_parameter("nv", [128, Wn], f16, isOutput=False)
    stateT = nc.declare_dram_parameter("stateT", [DIM, B], f16, isOutput=False)
    W1 = nc.declare_dram_parameter("W1", [3 * DIM, DIM], f16, isOutput=False)
    W2 = nc.declare_dram_parameter("W2", [DIM, DIM], f16, isOutput=False)
    W3 = nc.declare_dram_parameter("W3", [DIM, DIM], f16, isOutput=False)
    # vecs columns: b1,g1,be1,b2,g2,be2,b3,g3,be3
    vecs = nc.declare_dram_parameter("vecs", [DIM, 9], f32, isOutput=False)
    out = nc.declare_dram_parameter("out", [DIM, B], f32, isOutput=True)

    ag_in = [nc.dram_tensor(f"ag_in{h}", [2 * DIM, 4 * NH], f16)
             for h in range(2)]
    ag_out = [nc.dram_tensor(f"ag_out{h}", [2 * DIM * N_CORES, 4 * NH], f16,
                             addr_space="Shared") for h in range(2)]

    # greedy engine assignment for reduce pieces
    eng_time = [0.0, 0.0]

    def pick_engine(fd):
        costs = [eng_time[e] + _eng_cost(e, fd) for e in range(2)]
        e = int(np.argmin(costs))
        eng_time[e] = costs[e]
        return e

    # chunk index (in the edge stream) after which each half's quads are done
    def half_ready_chunk(pieces, h):
        lim = (h + 1) * NH
        return max((p[0] for p in pieces if p[3] < lim), default=-1)

    h1_chunk = half_ready_chunk(pieces_e, 0)

    with tile.TileContext(nc) as tc:
        with tc.tile_pool(name="echunks", bufs=3) as echunks, \
             tc.tile_pool(name="nchunks", bufs=1) as nchunks, \
             tc.tile_pool(name="const", bufs=1) as const, \
             tc.tile_pool(name="work", bufs=1) as work:

            # grouped means: cols 0..31 edge, 32..63 node (f32)
            sums2 = work.tile([128, 2 * NQ], f32, tag="sums2")
            nparts = 64
            parts = work.tile([128, nparts], f32, tag="parts")

            np_used = [0]
            pending = {}

            def emit_piece(ct, lo, hi, g, scol, pieces):
                npieces = sum(1 for p in pieces if p[3] == g)
                if npieces == 1:
                    dst = sums2[:, scol + g:scol + g + 1]
                else:
                    j = np_used[0]
                    np_used[0] += 1
                    dst = parts[:, j:j + 1]
                    pending.setdefault((scol, g), []).append(j)
                e = pick_engine(hi - lo)
                if e == 0:
                    nc.vector.tensor_reduce(
                        out=dst, in_=ct[:, lo:hi],
                        axis=mybir.AxisListType.X,
                        op=mybir.AluOpType.add)
                else:
                    # in-place copy: only accum_out matters
                    nc.scalar.activation(
                        out=ct[:, lo:hi], in_=ct[:, lo:hi],
                        func=mybir.ActivationFunctionType.Copy,
                        accum_out=dst)

            def flush_combines(scol, g_lo, g_hi):
                for (sc, g), js in list(pending.items()):
                    if sc != scol or not (g_lo <= g < g_hi):
                        continue
                    del pending[(sc, g)]
                    dst = sums2[:, sc + g:sc + g + 1]
                    nc.vector.tensor_tensor(dst, parts[:, js[0]:js[0] + 1],
                                            parts[:, js[1]:js[1] + 1],
                                            mybir.AluOpType.add)
                    for j in js[2:]:
                        nc.vector.tensor_tensor(dst, dst, parts[:, j:j + 1],
                                                mybir.AluOpType.add)

            def emit_gather(h):
                # cast+move sums2 for quads [h*NH, (h+1)*NH) of both streams
                # into ag_in[h], then AllGather
                for bq in range(4):
                    for strm in range(2):
                        src = sums2[32 * bq:32 * bq + DIM,
                                    NQ * strm + NH * h:NQ * strm + NH * h + NH]
                        dst = ag_in[h][DIM * strm:DIM * strm + DIM,
                                       16 * bq:16 * bq + NH]
                        nc.gpsimd.dma_start(out=dst, in_=src)
                nc.gpsimd.collective_compute(
                    "AllGather",
                    mybir.AluOpType.bypass,
                    replica_groups=[list(range(N_CORES))],
                    ins=[ag_in[h][:, :]],
                    outs=[ag_out[h][:, :]],
                )

            comb = work.tile([3 * DIM, B], f16, tag="comb")

            def emit_comb(h):
                agv = ag_out[h].rearrange("(r p) s -> r p s", p=2 * DIM)
                for r in range(N_CORES):
                    nc.sync.dma_start(
                        out=comb[0:2 * DIM, SEGS * r + 64 * h:
                                 SEGS * r + 64 * h + 64],
                        in_=agv[r])

            # ---- node stream first (small; its reduces hide under the
            # edge stream DMA) ----
            for ci, (c0, cw) in enumerate(chunks_n):
                ct = nchunks.tile([128, cw], f16, tag=f"nch{ci}")
                nc.sync.dma_start(out=ct, in_=nv[:, c0:c0 + cw])
                for (pci, lo, hi, g, nth) in pieces_n:
                    if pci == ci:
                        emit_piece(ct, lo, hi, g, NQ, pieces_n)
            flush_combines(NQ, 0, NQ)

            # ---- edge stream, with the half-1 gather dropped in as soon
            # as quads 0..15 are complete ----
            for ci, (c0, cw) in enumerate(chunks_e):
                ct = echunks.tile([128, cw], f16,
                                  tag="ech" if cw == CW else "echL")
                nc.sync.dma_start(out=ct, in_=ev[:, c0:c0 + cw])
                for (pci, lo, hi, g, nth) in pieces_e:
                    if pci == ci:
                        emit_piece(ct, lo, hi, g, 0, pieces_e)
                if ci == h1_chunk:
                    flush_combines(0, 0, NH)
                    emit_gather(0)
                    emit_comb(0)
            flush_combines(0, NH, NQ)
            emit_gather(1)
            emit_comb(1)

            nc.sync.dma_start(out=comb[2 * DIM:3 * DIM, :], in_=stateT[:, :])

            # ---- MLP with BatchNorm ([feat, graph] layout) ----
            w1s = const.tile([3 * DIM, DIM], f16)
            nc.sync.dma_start(out=w1s, in_=W1[:, :])
            w2s = const.tile([DIM, DIM], f16)
            nc.sync.dma_start(out=w2s, in_=W2[:, :])
            w3s = const.tile([DIM, DIM], f16)
            nc.sync.dma_start(out=w3s, in_=W3[:, :])
            vs = const.tile([DIM, 9], f32)
            nc.sync.dma_start(out=vs, in_=vecs[:, :])

            with tc.tile_pool(name="epsum", bufs=1, space="PSUM") as epsum:
                h = comb
                for layer in range(3):
                    w = (w1s, w2s, w3s)[layer]
                    bcol = vs[:, 3 * layer:3 * layer + 1]
                    gcol = vs[:, 3 * layer + 1:3 * layer + 2]
                    becol = vs[:, 3 * layer + 2:3 * layer + 3]

                    ps_h = epsum.tile([DIM, B], f32, tag="ps_h")
                    for half in range(2):
                        sl = slice(half * 512, (half + 1) * 512)
                        nc.tensor.matmul(out=ps_h[:, sl], lhsT=w[:, :],
                                         rhs=h[:, sl], start=True, stop=True)
                    hl = work.tile([DIM, B], f32, tag="hl")
                    func = (mybir.ActivationFunctionType.Relu if layer < 2
                            else mybir.ActivationFunctionType.Identity)
                    nc.scalar.activation(out=hl, in_=ps_h, func=func,
                                         bias=bcol)

                    # batchnorm over the free (graph) dim:
                    # var = E[h^2] - m^2, normalize via one mult+add pass.
                    # Sum(h) on Vector while Sum(h^2) runs on Scalar.
                    msum = work.tile([DIM, 1], f32, tag="msum")
                    nc.vector.tensor_reduce(out=msum, in_=hl,
                                            axis=mybir.AxisListType.X,
                                            op=mybir.AluOpType.add)
                    s2sum = work.tile([DIM, 1], f32, tag="s2sum")
                    sq = work.tile([DIM, B], f32, tag="sq")
                    nc.scalar.activation(
                        out=sq, in_=hl,
                        func=mybir.ActivationFunctionType.Square,
                        accum_out=s2sum)
                    m = work.tile([DIM, 1], f32, tag="m")
                    nc.scalar.mul(m, msum, 1.0 / B)
                    mm = work.tile([DIM, 1], f32, tag="mm")
                    nc.vector.tensor_tensor(mm, m, m, mybir.AluOpType.mult)
                    # veps = s2sum/B - m^2 + eps
                    veps0 = work.tile([DIM, 1], f32, tag="veps0")
                    nc.vector.tensor_scalar(veps0, mm, -1.0, EPS,
                                            mybir.AluOpType.mult,
                                            mybir.AluOpType.add)
                    veps = work.tile([DIM, 1], f32, tag="veps")
                    nc.vector.tensor_scalar(veps, s2sum, 1.0 / B, veps0,
                                            mybir.AluOpType.mult,
                                            mybir.AluOpType.add)
                    sd = work.tile([DIM, 1], f32, tag="sd")
                    nc.scalar.sqrt(sd, veps)
                    rstd = work.tile([DIM, 1], f32, tag="rstd")
                    nc.vector.reciprocal(rstd, sd)
                    rg = work.tile([DIM, 1], f32, tag="rg")
                    nc.vector.tensor_tensor(rg, rstd, gcol,
                                            mybir.AluOpType.mult)
                    # off = be - m * rg
                    off = work.tile([DIM, 1], f32, tag="off")
                    nc.vector.tensor_tensor(off, m, rg, mybir.AluOpType.mult)
                    nc.vector.tensor_tensor(off, becol, off,
                                            mybir.AluOpType.subtract)
                    odt = f16 if layer < 2 else f32
                    hb = work.tile([DIM, B], odt,
                                   tag="hb16" if layer < 2 else "hb32")
                    nc.vector.tensor_scalar(hb, hl, rg, off,
                                            mybir.AluOpType.mult,
                                            mybir.AluOpType.add)
                    h = hb

                nc.sync.dma_start(out=out[:, :], in_=h)

    nc.compile()
    return nc


def _pack_t(vals, seg, cnt, assign, rank_of, base, W_pad):
    """Scatter scaled fp16 rows into the transposed per-core layout
    [N_CORES, 128, W_pad] (partition 32*b + feat, column base[g] + i)."""
    order = np.argsort(seg, kind="stable")
    svals = vals[order]
    offs = np.zeros(B + 1, dtype=np.int64)
    np.cumsum(cnt, out=offs[1:])

    A = np.zeros((N_CORES, 4, DIM, W_pad), dtype=np.float16)
    for s in range(B):
        c = int(cnt[s])
        if c == 0:
            continue
        k = int(assign[s])
        r = int(rank_of[s])
        g, bq = r // 4, r % 4
        b0 = int(base[g])
        A[k, bq, :, b0:b0 + c] = svals[offs[s]:offs[s + 1]].T
    return A.reshape(N_CORES, 128, W_pad)


def run(inputs, trace=False, sim=False):
    x = np.asarray(inputs["x"], dtype=np.float32)
    edge_index = np.asarray(inputs["edge_index"]).astype(np.int64)
    edge_attr = np.asarray(inputs["edge_attr"], dtype=np.float32)
    state = np.asarray(inputs["state"], dtype=np.float32)
    batch = np.asarray(inputs["batch"]).astype(np.int64)

    eseg = batch[edge_index[0]]
    ecnt = np.bincount(eseg, minlength=B)
    ncnt = np.bincount(batch, minlength=B)

    assign, rank_of, gsched_e, gsched_n, p_global = _plan(ecnt, ncnt)
    base_e, We, chunks_e, pieces_e = _col_plan(gsched_e)
    base_n, Wn, chunks_n, pieces_n = _col_plan(gsched_n)

    # fold the scatter-mean division into the data, cast fp16
    recip_e = (1.0 / np.maximum(ecnt, 1)).astype(np.float32)
    recip_n = (1.0 / np.maximum(ncnt, 1)).astype(np.float32)
    evals = (edge_attr * recip_e[eseg][:, None]).astype(np.float16)
    nvals = (x * recip_n[batch][:, None]).astype(np.float16)

    ev = _pack_t(evals, eseg, ecnt, assign, rank_of, base_e, We)
    nv = _pack_t(nvals, batch, ncnt, assign, rank_of, base_n, Wn)

    vecs = np.stack([np.asarray(inputs[k], np.float32) for k in
                     ("b1", "g1", "be1", "b2", "g2", "be2", "b3", "g3", "be3")],
                    axis=1).astype(np.float32)  # [32, 9]

    shared = {
        "stateT": np.ascontiguousarray(state.T[:, p_global]).astype(np.float16),
        "W1": np.asarray(inputs["W1"], np.float16),
        "W2": np.asarray(inputs["W2"], np.float16),
        "W3": np.asarray(inputs["W3"], np.float16),
        "vecs": vecs,
    }
    in_maps = []
    for k in range(N_CORES):
        m = dict(shared)
        m["ev"] = np.ascontiguousarray(ev[k])
        m["nv"] = np.ascontiguousarray(nv[k])
        in_maps.append(m)

    key = (tuple(chunks_e), tuple(pieces_e), tuple(chunks_n), tuple(pieces_n))
    if key not in _CACHE:
        _CACHE[key] = _build_nc(((We, chunks_e, pieces_e),
                                 (Wn, chunks_n, pieces_n)))
    nc = _CACHE[key]

    if sim:
        from concourse.bass_interp import MultiCoreSim
        msim = MultiCoreSim(nc, num_cores=N_CORES)
        for c in range(N_CORES):
            cs = msim.cores[c]
            for kk, vv in in_maps[c].items():
                cs.tensor(kk)[:] = vv
        msim.simulate(check_with_hw=False)
        outT = np.array(msim.cores[0].tensor("out"))
        res = None
    else:
        res = run_bass_kernel_spmd(nc, in_maps, core_ids=list(range(N_CORES)),
                                   trace=trace)
        outT = res.results[0]["out"]  # [32, 1024] in permuted graph order

    outP = outT.T.astype(np.float32)          # [1024(perm), 32]
    outF = np.empty_like(outP)
    outF[p_global] = outP
    return np.ascontiguousarray(outF), res


def kernel(**inputs) -> np.ndarray:
    out, _ = run(inputs, trace=False)
    return out


# revision 16
# speedup vs baseline: 1.1169x; 1.1169x over previous
"""Trainium2 Bass kernel for nn_MEGNet_State_876173328941.

MEGNet state update: u_e = scatter_mean(edge_attr, batch[edge_index[0]], B),
u_v = scatter_mean(x, batch, B), comb = [u_e, u_v, state], then a 3-layer MLP
(96->32->32->32) with training-mode BatchNorm over the batch dim.

v4 design: transposed streaming layout, dual-engine free-dim reduction,
two-phase AllGather (first half hidden under the stream).
  - Host folds the 1/count division into the data, casts to fp16, and packs
    each core's stream TRANSPOSED: partition p = 32*b + feat where b is the
    graph's block within its quad (4 graphs per quad), free dim = row index.
    Graph rows are contiguous column ranges, zero-padded to a shared
    cross-core schedule.
  - Device streams [128, CW] fp16 chunks and segment-reduces along the free
    dim with Vector (tensor_reduce) and Scalar (activation accum_out) in
    parallel; greedy cost-balanced piece assignment. Node stream goes first
    so its many small reduces hide under the edge DMA.
  - Grouped means are cast+moved straight into the collective input with
    gpsimd SBUF->DRAM casting DMAs. Quads 0..15 AllGather mid-stream
    (hidden); quads 16..31 AllGather at the end (pays the ~20us CC floor
    once). The tiny MLP+BatchNorm runs redundantly on every core in
    [feat, graph] layout, with BatchNorm restructured as E[h^2]-m^2 so the
    Scalar engine stays on one activation table set.
"""

import sys

sys.path.insert(0, "/opt/trn_rl_repo")

import numpy as np

import concourse.bacc as bacc
import concourse.tile as tile
from concourse import mybir
from concourse.bass_utils import run_bass_kernel_spmd

DIM = 32
B = 1024
N_CORES = 8
SEGS = 128          # graphs per core
NQ = SEGS // 4      # quads (groups of 4 graphs) per core
NH = NQ // 2        # quads per gather phase
CW = 16384          # stream columns per DMA chunk
ALIGN = 64
EPS = 1e-5

_CACHE = {}


def _plan(ecnt, ncnt):
    """Balanced graph->core assignment plus shared per-quad column widths."""
    w = ecnt + ncnt

    order_desc = np.argsort(-w, kind="stable")
    load = np.zeros(N_CORES, dtype=np.int64)
    nseg = np.zeros(N_CORES, dtype=np.int64)
    assign = np.zeros(B, dtype=np.int64)
    for s in order_desc:
        open_cores = np.where(nseg < SEGS)[0]
        k = open_cores[np.argmin(load[open_cores])]
        assign[s] = k
        load[k] += w[s]
        nseg[k] += 1

    order = np.zeros((N_CORES, SEGS), dtype=np.int64)   # rank -> global seg
    rank_of = np.zeros(B, dtype=np.int64)
    for k in range(N_CORES):
        segs_k = np.where(assign == k)[0]
        segs_k = segs_k[np.argsort(-w[segs_k], kind="stable")]
        order[k] = segs_k
        rank_of[segs_k] = np.arange(SEGS)

    def gsched(cnt):
        c = cnt[order].reshape(N_CORES, NQ, 4)     # [core, quad, block]
        m = c.max(axis=(0, 2))                     # [NQ]
        return ((m + ALIGN - 1) // ALIGN * ALIGN).astype(np.int64)

    gsched_e = gsched(ecnt)
    gsched_n = gsched(ncnt)

    # gathered local col l = 64*half + 16*b + (q % 16) for rank r = 4*q + b
    p_global = np.zeros(N_CORES * SEGS, dtype=np.int64)
    for k in range(N_CORES):
        for r in range(SEGS):
            q, bq = r // 4, r % 4
            ll = 64 * (q // NH) + 16 * bq + (q % NH)
            p_global[k * SEGS + ll] = order[k, r]
    return assign, rank_of, gsched_e, gsched_n, p_global


def _col_plan(gs):
    """Column bases, padded width, and chunk-relative reduce pieces.

    Returns (base[NQ+1], W_pad, chunks, pieces) where chunks is a list of
    (col0, width) and pieces is a list of (chunk_idx, lo, hi, group, nth).
    """
    base = np.zeros(NQ + 1, dtype=np.int64)
    np.cumsum(gs, out=base[1:])
    W = int(base[-1])
    W_pad = (W + 511) // 512 * 512
    chunks = []
    c0 = 0
    while c0 < W_pad:
        cw = min(CW, W_pad - c0)
        chunks.append((c0, cw))
        c0 += cw
    pieces = []
    for g in range(NQ):
        lo, hi = int(base[g]), int(base[g + 1])
        nth = 0
        for ci, (c0, cw) in enumerate(chunks):
            a, b_ = max(lo, c0), min(hi, c0 + cw)
            if a < b_:
                pieces.append((ci, a - c0, b_ - c0, g, nth))
                nth += 1
        assert nth >= 1
    return base, W_pad, chunks, pieces


# modeled ns cost per reduce piece, per engine
def _eng_cost(eng, fd):
    if eng == 0:     # Vector (DVE)
        return (58 + fd) / 0.96
    return (352 + fd) / 1.2  # Scalar (ACT)


def _build_nc(plan_pack):
    (We, chunks_e, pieces_e), (Wn, chunks_n, pieces_n) = plan_pack
    nc = bacc.Bacc("TRN2", target_bir_lowering=False, debug=False,
                   enable_asserts=False, num_devices=N_CORES)
    f16 = mybir.dt.float16
    f32 = mybir.dt.float32

    ev = nc.declare_dram_parameter("ev", [128, We], f16, isOutput=False)
    nv = nc.declare_dram_parameter("nv", [128, Wn], f16, isOutput=False)
    stateT = nc.declare_dram_parameter("stateT", [DIM, B], f16, isOutput=False)
    W1 = nc.declare_dram_parameter("W1", [3 * DIM, DIM], f16, isOutput=False)
    W2 = nc.declare_dram_parameter("W2", [DIM, DIM], f16, isOutput=False)
    W3 = nc.declare_dram_parameter("W3", [DIM, DIM], f16, isOutput=False)
    # vecs columns: b1,g1,be1,b2,g2,be2,b3,g3,be3
    vecs = nc.declare_dram_parameter("vecs", [DIM, 9], f32, isOutput=False)
    out = nc.declare_dram_parameter("out", [DIM, B], f32, isOutput=True)

    ag_in = [nc.dram_tensor(f"ag_in{h}", [2 * DIM, 4 * NH], f16)
             for h in range(2)]
    ag_out = [nc.dram_tensor(f"ag_out{h}", [2 * DIM * N_CORES, 4 * NH], f16,
                             addr_space="Shared") for h in range(2)]

    # greedy engine assignment for reduce pieces
    eng_time = [0.0, 0.0]

    def pick_engine(fd):
        costs = [eng_time[e] + _eng_cost(e, fd) for e in range(2)]
        e = int(np.argmin(costs))
        eng_time[e] = costs[e]
        return e

    # chunk index (in the edge stream) after which each half's quads are done
    def half_ready_chunk(pieces, h):
        lim = (h + 1) * NH
        return max((p[0] for p in pieces if p[3] < lim), default=-1)

    h1_chunk = half_ready_chunk(pieces_e, 0)

    with tile.TileContext(nc) as tc:
        with tc.tile_pool(name="echunks", bufs=3) as echunks, \
             tc.tile_pool(name="nchunks", bufs=1) as nchunks, \
             tc.tile_pool(name="const", bufs=1) as const, \
             tc.tile_pool(name="work", bufs=1) as work:

            # grouped means: cols 0..31 edge, 32..63 node (f32)
            sums2 = work.tile([128, 2 * NQ], f32, tag="sums2")
            nparts = 64
            parts = work.tile([128, nparts], f32, tag="parts")

            np_used = [0]
            pending = {}

            def emit_piece(ct, lo, hi, g, scol, pieces):
                npieces = sum(1 for p in pieces if p[3] == g)
                if npieces == 1:
                    dst = sums2[:, scol + g:scol + g + 1]
                else:
                    j = np_used[0]
                    np_used[0] += 1
                    dst = parts[:, j:j + 1]
                    pending.setdefault((scol, g), []).append(j)
                e = pick_engine(hi - lo)
                if e == 0:
                    nc.vector.tensor_reduce(
                        out=dst, in_=ct[:, lo:hi],
                        axis=mybir.AxisListType.X,
                        op=mybir.AluOpType.add)
                else:
                    # in-place copy: only accum_out matters
                    nc.scalar.activation(
                        out=ct[:, lo:hi], in_=ct[:, lo:hi],
                        func=mybir.ActivationFunctionType.Copy,
                        accum_out=dst)

            def flush_combines(scol, g_lo, g_hi):
                for (sc, g), js in list(pending.items()):
                    if sc != scol or not (g_lo <= g < g_hi):
                        continue
                    del pending[(sc, g)]
                    dst = sums2[:, sc + g:sc + g + 1]
                    nc.vector.tensor_tensor(dst, parts[:, js[0]:js[0] + 1],
                                            parts[:, js[1]:js[1] + 1],
                                            mybir.AluOpType.add)
                    for j in js[2:]:
                        nc.vector.tensor_tensor(dst, dst, parts[:, j:j + 1],
                                                mybir.AluOpType.add)

            def emit_gather(h):
                # cast+move sums2 for quads [h*NH, (h+1)*NH) of both streams
                # into ag_in[h], then AllGather
                for bq in range(4):
                    for strm in range(2):
                        src = sums2[32 * bq:32 * bq + DIM,
                                    NQ * strm + NH * h:NQ * strm + NH * h + NH]
                        dst = ag_in[h][DIM * strm:DIM * strm + DIM,
                                       16 * bq:16 * bq + NH]
                        nc.gpsimd.dma_start(out=dst, in_=src)
                nc.gpsimd.collective_compute(
                    "AllGather",
                    mybir.AluOpType.bypass,
                    replica_groups=[list(range(N_CORES))],
                    ins=[ag_in[h][:, :]],
                    outs=[ag_out[h][:, :]],
                )

            comb = work.tile([3 * DIM, B], f16, tag="comb")

            def emit_comb(h):
                # gpsimd queue: must NOT sit on the sync queue, where the
                # wait on the collective would head-of-line-block the
                # still-streaming chunk DMAs
                agv = ag_out[h].rearrange("(r p) s -> r p s", p=2 * DIM)
                for r in range(N_CORES):
                    nc.gpsimd.dma_start(
                        out=comb[0:2 * DIM, SEGS * r + 64 * h:
                                 SEGS * r + 64 * h + 64],
                        in_=agv[r])

            # constants up front (tiny; lands during the DMA ramp)
            w1s = const.tile([3 * DIM, DIM], f16)
            nc.sync.dma_start(out=w1s, in_=W1[:, :])
            w2s = const.tile([DIM, DIM], f16)
            nc.sync.dma_start(out=w2s, in_=W2[:, :])
            w3s = const.tile([DIM, DIM], f16)
            nc.sync.dma_start(out=w3s, in_=W3[:, :])
            vs = const.tile([DIM, 9], f32)
            nc.sync.dma_start(out=vs, in_=vecs[:, :])
            nc.sync.dma_start(out=comb[2 * DIM:3 * DIM, :], in_=stateT[:, :])

            # ---- node stream first (small; its reduces hide under the
            # edge stream DMA) ----
            for ci, (c0, cw) in enumerate(chunks_n):
                ct = nchunks.tile([128, cw], f16, tag=f"nch{ci}")
                nc.sync.dma_start(out=ct, in_=nv[:, c0:c0 + cw])
                for (pci, lo, hi, g, nth) in pieces_n:
                    if pci == ci:
                        emit_piece(ct, lo, hi, g, NQ, pieces_n)
            flush_combines(NQ, 0, NQ)

            # ---- edge stream, with the half-1 gather dropped in as soon
            # as quads 0..15 are complete ----
            for ci, (c0, cw) in enumerate(chunks_e):
                ct = echunks.tile([128, cw], f16,
                                  tag="ech" if cw == CW else "echL")
                nc.sync.dma_start(out=ct, in_=ev[:, c0:c0 + cw])
                for (pci, lo, hi, g, nth) in pieces_e:
                    if pci == ci:
                        emit_piece(ct, lo, hi, g, 0, pieces_e)
                if ci == h1_chunk:
                    flush_combines(0, 0, NH)
                    emit_gather(0)
                    emit_comb(0)
            flush_combines(0, NH, NQ)
            emit_gather(1)
            emit_comb(1)

            # ---- MLP with BatchNorm ([feat, graph] layout) ----
            with tc.tile_pool(name="epsum", bufs=1, space="PSUM") as epsum:
                h = comb
                for layer in range(3):
                    w = (w1s, w2s, w3s)[layer]
                    bcol = vs[:, 3 * layer:3 * layer + 1]
                    gcol = vs[:, 3 * layer + 1:3 * layer + 2]
                    becol = vs[:, 3 * layer + 2:3 * layer + 3]

                    ps_h = epsum.tile([DIM, B], f32, tag="ps_h")
                    for half in range(2):
                        sl = slice(half * 512, (half + 1) * 512)
                        nc.tensor.matmul(out=ps_h[:, sl], lhsT=w[:, :],
                                         rhs=h[:, sl], start=True, stop=True)
                    hl = work.tile([DIM, B], f32, tag="hl")
                    func = (mybir.ActivationFunctionType.Relu if layer < 2
                            else mybir.ActivationFunctionType.Identity)
                    nc.scalar.activation(out=hl, in_=ps_h, func=func,
                                         bias=bcol)

                    # batchnorm over the free (graph) dim:
                    # var = E[h^2] - m^2, normalize via one mult+add pass.
                    # Sum(h) on Vector while Sum(h^2) runs on Scalar.
                    msum = work.tile([DIM, 1], f32, tag="msum")
                    nc.vector.tensor_reduce(out=msum, in_=hl,
                                            axis=mybir.AxisListType.X,
                                            op=mybir.AluOpType.add)
                    s2sum = work.tile([DIM, 1], f32, tag="s2sum")
                    sq = work.tile([DIM, B], f32, tag="sq")
                    nc.scalar.activation(
                        out=sq, in_=hl,
                        func=mybir.ActivationFunctionType.Square,
                        accum_out=s2sum)
                    m = work.tile([DIM, 1], f32, tag="m")
                    nc.scalar.mul(m, msum, 1.0 / B)
                    mm = work.tile([DIM, 1], f32, tag="mm")
                    nc.vector.tensor_tensor(mm, m, m, mybir.AluOpType.mult)
                    # veps = s2sum/B - m^2 + eps
                    veps0 = work.tile([DIM, 1], f32, tag="veps0")
                    nc.vector.tensor_scalar(veps0, mm, -1.0, EPS,
                                            mybir.AluOpType.mult,
                                            mybir.AluOpType.add)
                    veps = work.tile([DIM, 1], f32, tag="veps")
                    nc.vector.tensor_scalar(veps, s2sum, 1.0 / B, veps0,
                                            mybir.AluOpType.mult,
                                            mybir.AluOpType.add)
                    sd = work.tile([DIM, 1], f32, tag="sd")
                    nc.scalar.sqrt(sd, veps)
                    rstd = work.tile([DIM, 1], f32, tag="rstd")
                    nc.vector.reciprocal(rstd, sd)
                    rg = work.tile([DIM, 1], f32, tag="rg")
                    nc.vector.tensor_tensor(rg, rstd, gcol,
                                            mybir.AluOpType.mult)
                    # off = be - m * rg
                    off = work.tile([DIM, 1], f32, tag="off")
                    nc.vector.tensor_tensor(off, m, rg, mybir.AluOpType.mult)
                    nc.vector.tensor_tensor(off, becol, off,
                                            mybir.AluOpType.subtract)
                    odt = f16 if layer < 2 else f32
                    hb = work.tile([DIM, B], odt,
                                   tag="hb16" if layer < 2 else "hb32")
                    nc.vector.tensor_scalar(hb, hl, rg, off,
                                            mybir.AluOpType.mult,
                                            mybir.AluOpType.add)
                    h = hb

                nc.sync.dma_start(out=out[:, :], in_=h)

    nc.compile()
    return nc


def _pack_t(vals, seg, cnt, assign, rank_of, base, W_pad):
    """Scatter scaled fp16 rows into the transposed per-core layout
    [N_CORES, 128, W_pad] (partition 32*b + feat, column base[g] + i)."""
    order = np.argsort(seg, kind="stable")
    svals = vals[order]
    offs = np.zeros(B + 1, dtype=np.int64)
    np.cumsum(cnt, out=offs[1:])

    A = np.zeros((N_CORES, 4, DIM, W_pad), dtype=np.float16)
    for s in range(B):
        c = int(cnt[s])
        if c == 0:
            continue
        k = int(assign[s])
        r = int(rank_of[s])
        g, bq = r // 4, r % 4
        b0 = int(base[g])
        A[k, bq, :, b0:b0 + c] = svals[offs[s]:offs[s + 1]].T
    return A.reshape(N_CORES, 128, W_pad)


def run(inputs, trace=False, sim=False):
    x = np.asarray(inputs["x"], dtype=np.float32)
    edge_index = np.asarray(inputs["edge_index"]).astype(np.int64)
    edge_attr = np.asarray(inputs["edge_attr"], dtype=np.float32)
    state = np.asarray(inputs["state"], dtype=np.float32)
    batch = np.asarray(inputs["batch"]).astype(np.int64)

    eseg = batch[edge_index[0]]
    ecnt = np.bincount(eseg, minlength=B)
    ncnt = np.bincount(batch, minlength=B)

    assign, rank_of, gsched_e, gsched_n, p_global = _plan(ecnt, ncnt)
    base_e, We, chunks_e, pieces_e = _col_plan(gsched_e)
    base_n, Wn, chunks_n, pieces_n = _col_plan(gsched_n)

    # fold the scatter-mean division into the data, cast fp16
    recip_e = (1.0 / np.maximum(ecnt, 1)).astype(np.float32)
    recip_n = (1.0 / np.maximum(ncnt, 1)).astype(np.float32)
    evals = (edge_attr * recip_e[eseg][:, None]).astype(np.float16)
    nvals = (x * recip_n[batch][:, None]).astype(np.float16)

    ev = _pack_t(evals, eseg, ecnt, assign, rank_of, base_e, We)
    nv = _pack_t(nvals, batch, ncnt, assign, rank_of, base_n, Wn)

    vecs = np.stack([np.asarray(inputs[k], np.float32) for k in
                     ("b1", "g1", "be1", "b2", "g2", "be2", "b3", "g3", "be3")],
                    axis=1).astype(np.float32)  # [32, 9]

    shared = {
        "stateT": np.ascontiguousarray(state.T[:, p_global]).astype(np.float16),
        "W1": np.asarray(inputs["W1"], np.float16),
        "W2": np.asarray(inputs["W2"], np.float16),
        "W3": np.asarray(inputs["W3"], np.float16),
        "vecs": vecs,
    }
    in_maps = []
    for k in range(N_CORES):
        m = dict(shared)
        m["ev"] = np.ascontiguousarray(ev[k])
        m["nv"] = np.ascontiguousarray(nv[k])
        in_maps.append(m)

    key = (tuple(chunks_e), tuple(pieces_e), tuple(chunks_n), tuple(pieces_n))
    if key not in _CACHE:
        _CACHE[key] = _build_nc(((We, chunks_e, pieces_e),
                                 (Wn, chunks_n, pieces_n)))
    nc = _CACHE[key]

    if sim:
        from concourse.bass_interp import MultiCoreSim
        msim = MultiCoreSim(nc, num_cores=N_CORES)
        for c in range(N_CORES):
            cs = msim.cores[c]
            for kk, vv in in_maps[c].items():
                cs.tensor(kk)[:] = vv
        msim.simulate(check_with_hw=False)
        outT = np.array(msim.cores[0].tensor("out"))
        res = None
    else:
        res = run_bass_kernel_spmd(nc, in_maps, core_ids=list(range(N_CORES)),
                                   trace=trace)
        outT = res.results[0]["out"]  # [32, 1024] in permuted graph order

    outP = outT.T.astype(np.float32)          # [1024(perm), 32]
    outF = np.empty_like(outP)
    outF[p_global] = outP
    return np.ascontiguousarray(outF), res


def kernel(**inputs) -> np.ndarray:
    out, _ = run(inputs, trace=False)
    return out


# revision 20
# speedup vs baseline: 1.1942x; 1.0692x over previous
"""Trainium2 Bass kernel for nn_MEGNet_State_876173328941.

MEGNet state update: u_e = scatter_mean(edge_attr, batch[edge_index[0]], B),
u_v = scatter_mean(x, batch, B), comb = [u_e, u_v, state], then a 3-layer MLP
(96->32->32->32) with training-mode BatchNorm over the batch dim.

v5 design: transposed streaming layout, dual-engine free-dim reduction,
warmed-up single AllGather, bn_stats BatchNorm.
  - Host folds the 1/count division into the data, casts to fp16, and packs
    each core's stream TRANSPOSED: partition p = 32*b + feat where b is the
    graph's block within its quad (4 graphs per quad), free dim = row index.
    Graph rows are contiguous column ranges, zero-padded to a shared
    cross-core schedule.
  - Device streams [128, CW] fp16 chunks and segment-reduces along the free
    dim with Vector (tensor_reduce) and Scalar (activation accum_out) in
    parallel; greedy cost-balanced piece assignment. Node stream goes first
    so its many small reduces hide under the edge DMA.
  - A dummy collective at kernel start absorbs the collective first-call
    cost (NRT staging) under the DMA ramp; the real 16KB AllGather then
    runs warm (~7-10us). A dummy sqrt preloads the Scalar activation table
    set so no table switches happen in the tail.
  - BatchNorm uses bn_stats/bn_aggr (exact mean + biased var in one Vector
    pass pair) and a single mult+add normalization pass.
"""

import sys

sys.path.insert(0, "/opt/trn_rl_repo")

import numpy as np

import concourse.bacc as bacc
import concourse.tile as tile
from concourse import mybir
from concourse.bass_utils import run_bass_kernel_spmd

DIM = 32
B = 1024
N_CORES = 8
SEGS = 128          # graphs per core
NQ = SEGS // 4      # quads (groups of 4 graphs) per core
CW = 16384          # stream columns per DMA chunk
ALIGN = 64
EPS = 1e-5

_CACHE = {}


def _plan(ecnt, ncnt):
    """Balanced graph->core assignment plus shared per-quad column widths."""
    w = ecnt + ncnt

    order_desc = np.argsort(-w, kind="stable")
    load = np.zeros(N_CORES, dtype=np.int64)
    nseg = np.zeros(N_CORES, dtype=np.int64)
    assign = np.zeros(B, dtype=np.int64)
    for s in order_desc:
        open_cores = np.where(nseg < SEGS)[0]
        k = open_cores[np.argmin(load[open_cores])]
        assign[s] = k
        load[k] += w[s]
        nseg[k] += 1

    order = np.zeros((N_CORES, SEGS), dtype=np.int64)   # rank -> global seg
    rank_of = np.zeros(B, dtype=np.int64)
    for k in range(N_CORES):
        segs_k = np.where(assign == k)[0]
        segs_k = segs_k[np.argsort(-w[segs_k], kind="stable")]
        order[k] = segs_k
        rank_of[segs_k] = np.arange(SEGS)

    def gsched(cnt):
        c = cnt[order].reshape(N_CORES, NQ, 4)     # [core, quad, block]
        m = c.max(axis=(0, 2))                     # [NQ]
        return ((m + ALIGN - 1) // ALIGN * ALIGN).astype(np.int64)

    gsched_e = gsched(ecnt)
    gsched_n = gsched(ncnt)

    # gathered local col l = 32*b + q  for rank r = 4*q + b
    p_global = np.zeros(N_CORES * SEGS, dtype=np.int64)
    for k in range(N_CORES):
        for r in range(SEGS):
            q, bq = r // 4, r % 4
            p_global[k * SEGS + 32 * bq + q] = order[k, r]
    return assign, rank_of, gsched_e, gsched_n, p_global


def _col_plan(gs):
    """Column bases, padded width, and chunk-relative reduce pieces."""
    base = np.zeros(NQ + 1, dtype=np.int64)
    np.cumsum(gs, out=base[1:])
    W = int(base[-1])
    W_pad = (W + 511) // 512 * 512
    chunks = []
    c0 = 0
    while c0 < W_pad:
        cw = min(CW, W_pad - c0)
        chunks.append((c0, cw))
        c0 += cw
    pieces = []
    for g in range(NQ):
        lo, hi = int(base[g]), int(base[g + 1])
        nth = 0
        for ci, (c0, cw) in enumerate(chunks):
            a, b_ = max(lo, c0), min(hi, c0 + cw)
            if a < b_:
                pieces.append((ci, a - c0, b_ - c0, g, nth))
                nth += 1
        assert nth >= 1
    return base, W_pad, chunks, pieces


# modeled ns cost per reduce piece, per engine
def _eng_cost(eng, fd):
    if eng == 0:     # Vector (DVE)
        return (58 + fd) / 0.96
    return (352 + fd) / 1.2  # Scalar (ACT)


def _build_nc(plan_pack):
    (We, chunks_e, pieces_e), (Wn, chunks_n, pieces_n) = plan_pack
    nc = bacc.Bacc("TRN2", target_bir_lowering=False, debug=False,
                   enable_asserts=False, num_devices=N_CORES)
    f16 = mybir.dt.float16
    f32 = mybir.dt.float32

    ev = nc.declare_dram_parameter("ev", [128, We], f16, isOutput=False)
    nv = nc.declare_dram_parameter("nv", [128, Wn], f16, isOutput=False)
    stateT = nc.declare_dram_parameter("stateT", [DIM, B], f16, isOutput=False)
    W1 = nc.declare_dram_parameter("W1", [3 * DIM, DIM], f16, isOutput=False)
    W2 = nc.declare_dram_parameter("W2", [DIM, DIM], f16, isOutput=False)
    W3 = nc.declare_dram_parameter("W3", [DIM, DIM], f16, isOutput=False)
    # vecs columns: b1,g1,be1,b2,g2,be2,b3,g3,be3
    vecs = nc.declare_dram_parameter("vecs", [DIM, 9], f32, isOutput=False)
    out = nc.declare_dram_parameter("out", [DIM, B], f32, isOutput=True)

    ag_in = nc.dram_tensor("ag_in", [2 * DIM, SEGS], f16)
    ag_out = nc.dram_tensor("ag_out", [2 * DIM * N_CORES, SEGS], f16,
                            addr_space="Shared")
    agw_in = nc.dram_tensor("agw_in", [DIM, 8], f16)
    agw_out = nc.dram_tensor("agw_out", [DIM * N_CORES, 8], f16,
                             addr_space="Shared")

    eng_time = [0.0, 0.0]

    def pick_engine(fd):
        costs = [eng_time[e] + _eng_cost(e, fd) for e in range(2)]
        e = int(np.argmin(costs))
        eng_time[e] = costs[e]
        return e

    with tile.TileContext(nc) as tc:
        with tc.tile_pool(name="echunks", bufs=3) as echunks, \
             tc.tile_pool(name="nchunks", bufs=1) as nchunks, \
             tc.tile_pool(name="const", bufs=1) as const, \
             tc.tile_pool(name="work", bufs=1) as work:

            # ---- warmups: collective first-call cost + ACT sqrt table ----
            wz = const.tile([DIM, 8], f16)
            nc.vector.memset(wz, 0.0)
            nc.sync.dma_start(out=agw_in[:, :], in_=wz)
            nc.gpsimd.collective_compute(
                "AllGather",
                mybir.AluOpType.bypass,
                replica_groups=[list(range(N_CORES))],
                ins=[agw_in[:, :]],
                outs=[agw_out[:, :]],
            )
            epsb = const.tile([DIM, 1], f32)
            nc.vector.memset(epsb, EPS)
            wq = const.tile([1, 1], f32)
            nc.vector.memset(wq, 1.0)
            wq2 = const.tile([1, 1], f32)
            nc.scalar.activation(out=wq2, in_=wq,
                                 func=mybir.ActivationFunctionType.Sqrt,
                                 bias=epsb[0:1, :])

            # constants up front (tiny; lands during the DMA ramp)
            comb = work.tile([3 * DIM, B], f16, tag="comb")
            w1s = const.tile([3 * DIM, DIM], f16)
            nc.sync.dma_start(out=w1s, in_=W1[:, :])
            w2s = const.tile([DIM, DIM], f16)
            nc.sync.dma_start(out=w2s, in_=W2[:, :])
            w3s = const.tile([DIM, DIM], f16)
            nc.sync.dma_start(out=w3s, in_=W3[:, :])
            vs = const.tile([DIM, 9], f32)
            nc.sync.dma_start(out=vs, in_=vecs[:, :])
            nc.sync.dma_start(out=comb[2 * DIM:3 * DIM, :], in_=stateT[:, :])

            # grouped means: cols 0..31 edge, 32..63 node (f32)
            sums2 = work.tile([128, 2 * NQ], f32, tag="sums2")
            nparts = 64
            parts = work.tile([128, nparts], f32, tag="parts")

            np_used = [0]
            pending = {}

            def emit_piece(ct, lo, hi, g, scol, pieces):
                npieces = sum(1 for p in pieces if p[3] == g)
                if npieces == 1:
                    dst = sums2[:, scol + g:scol + g + 1]
                else:
                    j = np_used[0]
                    np_used[0] += 1
                    dst = parts[:, j:j + 1]
                    pending.setdefault((scol, g), []).append(j)
                e = pick_engine(hi - lo)
                if e == 0:
                    nc.vector.tensor_reduce(
                        out=dst, in_=ct[:, lo:hi],
                        axis=mybir.AxisListType.X,
                        op=mybir.AluOpType.add)
                else:
                    # in-place copy: only accum_out matters
                    nc.scalar.activation(
                        out=ct[:, lo:hi], in_=ct[:, lo:hi],
                        func=mybir.ActivationFunctionType.Copy,
                        accum_out=dst)

            def flush_combines():
                for (sc, g), js in list(pending.items()):
                    del pending[(sc, g)]
                    dst = sums2[:, sc + g:sc + g + 1]
                    nc.vector.tensor_tensor(dst, parts[:, js[0]:js[0] + 1],
                                            parts[:, js[1]:js[1] + 1],
                                            mybir.AluOpType.add)
                    for j in js[2:]:
                        nc.vector.tensor_tensor(dst, dst, parts[:, j:j + 1],
                                                mybir.AluOpType.add)

            # ---- node stream first (small; its reduces hide under the
            # edge stream DMA) ----
            for ci, (c0, cw) in enumerate(chunks_n):
                ct = nchunks.tile([128, cw], f16, tag=f"nch{ci}")
                nc.sync.dma_start(out=ct, in_=nv[:, c0:c0 + cw])
                for (pci, lo, hi, g, nth) in pieces_n:
                    if pci == ci:
                        emit_piece(ct, lo, hi, g, NQ, pieces_n)

            # ---- edge stream ----
            for ci, (c0, cw) in enumerate(chunks_e):
                ct = echunks.tile([128, cw], f16,
                                  tag="ech" if cw == CW else "echL")
                nc.sync.dma_start(out=ct, in_=ev[:, c0:c0 + cw])
                for (pci, lo, hi, g, nth) in pieces_e:
                    if pci == ci:
                        emit_piece(ct, lo, hi, g, 0, pieces_e)
            flush_combines()

            # ---- gather: cast, un-group into ag_in, AllGather (warm) ----
            sums16 = work.tile([128, 2 * NQ], f16, tag="sums16")
            nc.vector.tensor_copy(sums16, sums2)
            for strm in range(2):
                for bq in range(4):
                    nc.sync.dma_start(
                        out=ag_in[DIM * strm:DIM * strm + DIM,
                                  NQ * bq:NQ * bq + NQ],
                        in_=sums16[32 * bq:32 * bq + DIM,
                                   NQ * strm:NQ * strm + NQ])
            nc.gpsimd.collective_compute(
                "AllGather",
                mybir.AluOpType.bypass,
                replica_groups=[list(range(N_CORES))],
                ins=[ag_in[:, :]],
                outs=[ag_out[:, :]],
            )
            agp = ag_out.rearrange("(r p) s -> p r s", p=2 * DIM)
            nc.sync.dma_start(out=comb[0:2 * DIM, :], in_=agp)

            # ---- MLP with BatchNorm ([feat, graph] layout) ----
            with tc.tile_pool(name="epsum", bufs=1, space="PSUM") as epsum:
                h = comb
                for layer in range(3):
                    w = (w1s, w2s, w3s)[layer]
                    bcol = vs[:, 3 * layer:3 * layer + 1]
                    gcol = vs[:, 3 * layer + 1:3 * layer + 2]
                    becol = vs[:, 3 * layer + 2:3 * layer + 3]

                    ps_h = epsum.tile([DIM, B], f32, tag="ps_h")
                    for half in range(2):
                        sl = slice(half * 512, (half + 1) * 512)
                        nc.tensor.matmul(out=ps_h[:, sl], lhsT=w[:, :],
                                         rhs=h[:, sl], start=True, stop=True)
                    hl = work.tile([DIM, B], f32, tag="hl")
                    func = (mybir.ActivationFunctionType.Relu if layer < 2
                            else mybir.ActivationFunctionType.Identity)
                    nc.scalar.activation(out=hl, in_=ps_h, func=func,
                                         bias=bcol)

                    # batchnorm over the free (graph) dim via bn_stats
                    st = work.tile([DIM, 12], f32, tag="st")
                    nc.vector.bn_stats(st[:, 0:6], hl[:, 0:512])
                    nc.vector.bn_stats(st[:, 6:12], hl[:, 512:1024])
                    ag = work.tile([DIM, 2], f32, tag="ag")
                    nc.vector.bn_aggr(ag, st)
                    # sd = sqrt(var + eps); rstd = 1/sd; rg = g*rstd;
                    # off = be - m*rg; hb = hl*rg + off
                    sd = work.tile([DIM, 1], f32, tag="sd")
                    nc.scalar.activation(out=sd, in_=ag[:, 1:2],
                                         func=mybir.ActivationFunctionType.Sqrt,
                                         bias=epsb[:, :])
                    rstd = work.tile([DIM, 1], f32, tag="rstd")
                    nc.vector.reciprocal(rstd, sd)
                    rg = work.tile([DIM, 1], f32, tag="rg")
                    nc.vector.tensor_tensor(rg, rstd, gcol,
                                            mybir.AluOpType.mult)
                    off = work.tile([DIM, 1], f32, tag="off")
                    nc.vector.tensor_tensor(off, ag[:, 0:1], rg,
                                            mybir.AluOpType.mult)
                    nc.vector.tensor_tensor(off, becol, off,
                                            mybir.AluOpType.subtract)
                    odt = f16 if layer < 2 else f32
                    hb = work.tile([DIM, B], odt,
                                   tag="hb16" if layer < 2 else "hb32")
                    nc.vector.tensor_scalar(hb, hl, rg, off,
                                            mybir.AluOpType.mult,
                                            mybir.AluOpType.add)
                    h = hb

                nc.sync.dma_start(out=out[:, :], in_=h)

    nc.compile()
    return nc


def _pack_t(vals, seg, cnt, assign, rank_of, base, W_pad):
    """Scatter scaled fp16 rows into the transposed per-core layout
    [N_CORES, 128, W_pad] (partition 32*b + feat, column base[g] + i)."""
    order = np.argsort(seg, kind="stable")
    svals = vals[order]
    offs = np.zeros(B + 1, dtype=np.int64)
    np.cumsum(cnt, out=offs[1:])

    A = np.zeros((N_CORES, 4, DIM, W_pad), dtype=np.float16)
    for s in range(B):
        c = int(cnt[s])
        if c == 0:
            continue
        k = int(assign[s])
        r = int(rank_of[s])
        g, bq = r // 4, r % 4
        b0 = int(base[g])
        A[k, bq, :, b0:b0 + c] = svals[offs[s]:offs[s + 1]].T
    return A.reshape(N_CORES, 128, W_pad)


def run(inputs, trace=False, sim=False):
    x = np.asarray(inputs["x"], dtype=np.float32)
    edge_index = np.asarray(inputs["edge_index"]).astype(np.int64)
    edge_attr = np.asarray(inputs["edge_attr"], dtype=np.float32)
    state = np.asarray(inputs["state"], dtype=np.float32)
    batch = np.asarray(inputs["batch"]).astype(np.int64)

    eseg = batch[edge_index[0]]
    ecnt = np.bincount(eseg, minlength=B)
    ncnt = np.bincount(batch, minlength=B)

    assign, rank_of, gsched_e, gsched_n, p_global = _plan(ecnt, ncnt)
    base_e, We, chunks_e, pieces_e = _col_plan(gsched_e)
    base_n, Wn, chunks_n, pieces_n = _col_plan(gsched_n)

    # fold the scatter-mean division into the data, cast fp16
    recip_e = (1.0 / np.maximum(ecnt, 1)).astype(np.float32)
    recip_n = (1.0 / np.maximum(ncnt, 1)).astype(np.float32)
    evals = (edge_attr * recip_e[eseg][:, None]).astype(np.float16)
    nvals = (x * recip_n[batch][:, None]).astype(np.float16)

    ev = _pack_t(evals, eseg, ecnt, assign, rank_of, base_e, We)
    nv = _pack_t(nvals, batch, ncnt, assign, rank_of, base_n, Wn)

    vecs = np.stack([np.asarray(inputs[k], np.float32) for k in
                     ("b1", "g1", "be1", "b2", "g2", "be2", "b3", "g3", "be3")],
                    axis=1).astype(np.float32)  # [32, 9]

    shared = {
        "stateT": np.ascontiguousarray(state.T[:, p_global]).astype(np.float16),
        "W1": np.asarray(inputs["W1"], np.float16),
        "W2": np.asarray(inputs["W2"], np.float16),
        "W3": np.asarray(inputs["W3"], np.float16),
        "vecs": vecs,
    }
    in_maps = []
    for k in range(N_CORES):
        m = dict(shared)
        m["ev"] = np.ascontiguousarray(ev[k])
        m["nv"] = np.ascontiguousarray(nv[k])
        in_maps.append(m)

    key = (tuple(chunks_e), tuple(pieces_e), tuple(chunks_n), tuple(pieces_n))
    if key not in _CACHE:
        _CACHE[key] = _build_nc(((We, chunks_e, pieces_e),
                                 (Wn, chunks_n, pieces_n)))
    nc = _CACHE[key]

    if sim:
        from concourse.bass_interp import MultiCoreSim
        msim = MultiCoreSim(nc, num_cores=N_CORES)
        for c in range(N_CORES):
            cs = msim.cores[c]
            for kk, vv in in_maps[c].items():
                cs.tensor(kk)[:] = vv
        msim.simulate(check_with_hw=False)
        outT = np.array(msim.cores[0].tensor("out"))
        res = None
    else:
        res = run_bass_kernel_spmd(nc, in_maps, core_ids=list(range(N_CORES)),
                                   trace=trace)
        outT = res.results[0]["out"]  # [32, 1024] in permuted graph order

    outP = outT.T.astype(np.float32)          # [1024(perm), 32]
    outF = np.empty_like(outP)
    outF[p_global] = outP
    return np.ascontiguousarray(outF), res


def kernel(**inputs) -> np.ndarray:
    out, _ = run(inputs, trace=False)
    return out
